# revision 2
# baseline (speedup 1.0000x reference)
"""HORN recurrent network kernel for 8x Trainium2 NeuronCores.

Model (T=512, B=256, I=128, N=1024, O=10):
    u_t  = i2h(batch_t)                          # input projection
    rec  = (1/sqrt(N)) * (y @ h2h_w.T + h2h_b)
    y'   = y + 0.1*(tanh(u_t + rec) - x - 0.2*y)
    x'   = x + 0.1*y'
    out  = x_T @ h2o_w.T + h2o_b

Sharding: data-parallel over batch, 32 per core; weights replicated.

Per-core implementation notes:
  - State kept in "n-layout" tiles [128, 256]: partition p / column 32*c+b
    holds element n = 128*c + p of batch-lane b.  The h2h matmul uses the
    state slices directly as the stationary operand (yT), streaming the
    (host-pre-laid-out) g*h2h_w.T as the moving operand in float32r
    (TF32-like: 11 mantissa bits, exact MAC) at 1 col/cycle.
  - The tanh argument is accumulated fully in PSUM per 512-wide bank from
    three matmul groups: a K=1 bias matmul (ones x (i2h_b + g*h2h_b)), the
    inlined input projection (batch kept SBUF-resident for all 512 steps,
    stationary [128,32] slice per step), and 8 h2h chunk matmuls.
  - ACT applies tanh straight out of PSUM (b-layout [32,512]); PE
    transposes the tanh output back to n-layout ([32,128] -> [128,32] x8
    into one PSUM bank, start/stop flags shared across the 8 transposes);
    DVE finishes the state update with 3 fused scalar_tensor_tensor ops.
  - x is carried as w = 0.1*x (fits the (in0 op0 scalar) op1 in1 op form);
    the final x = 10*w rescale is folded into the h2o weights host-side.
"""
import math
from contextlib import ExitStack

import numpy as np

import concourse.bacc as bacc
import concourse.bass_utils as bass_utils
import concourse.mybir as mybir
import concourse.tile as tile

T, B, I, N, O = 512, 256, 128, 1024, 10
H_STEP = 0.1
ALPHA = 1.0
OMEGA = 1.0
GAMMA = 0.1
GAIN_REC = 1.0 / math.sqrt(N)

NCORES = 8
BL = B // NCORES          # 32 batch lanes per core
NCH = N // 128            # 8 contraction chunks
DT = mybir.dt

_cache = {}


def build(n_steps=T):
    nc = bacc.Bacc("TRN2", target_bir_lowering=False, debug=False,
                   num_devices=NCORES)

    batcht_d = nc.dram_tensor("batcht", [I, n_steps * BL], DT.float32, kind="ExternalInput").ap()
    wg_d = nc.dram_tensor("wg", [128, NCH * N], DT.float32, kind="ExternalInput").ap()
    i2ht_d = nc.dram_tensor("i2ht", [I, N], DT.float32, kind="ExternalInput").ap()
    crow_d = nc.dram_tensor("crow", [1, N], DT.float32, kind="ExternalInput").ap()
    h2ot_d = nc.dram_tensor("h2ot", [128, NCH * O], DT.float32, kind="ExternalInput").ap()
    h2ob_d = nc.dram_tensor("h2ob", [1, O], DT.float32, kind="ExternalInput").ap()
    ident_d = nc.dram_tensor("ident", [32, 32], DT.float32, kind="ExternalInput").ap()
    out_d = nc.dram_tensor("out", [BL, O], DT.float32, kind="ExternalOutput").ap()

    AL = mybir.AluOpType
    TANH = mybir.ActivationFunctionType.Tanh

    with tile.TileContext(nc) as tc, ExitStack() as ctx:
        cpool = ctx.enter_context(tc.tile_pool(name="consts", bufs=1))
        spool = ctx.enter_context(tc.tile_pool(name="state", bufs=1))
        thpool = ctx.enter_context(tc.tile_pool(name="th", bufs=2))
        apool = ctx.enter_context(tc.tile_pool(name="aps", bufs=2, space="PSUM"))
        ttpool = ctx.enter_context(tc.tile_pool(name="tt", bufs=2, space="PSUM"))
        opool = ctx.enter_context(tc.tile_pool(name="ops", bufs=1, space="PSUM"))

        batcht_r = cpool.tile([I, n_steps * BL], DT.float32r)
        wg_r = cpool.tile([128, NCH * N], DT.float32r)
        i2ht_r = cpool.tile([I, N], DT.float32r)
        crow_r = cpool.tile([1, N], DT.float32r)
        ones_r = cpool.tile([1, BL], DT.float32r)
        ones_f = cpool.tile([1, BL], DT.float32)
        ident = cpool.tile([32, 32], DT.float32)
        h2ot = cpool.tile([128, NCH * O], DT.float32)
        h2ob_sb = cpool.tile([1, O], DT.float32)

        nc.sync.dma_start(ident[:], ident_d[:])
        nc.sync.dma_start(h2ot[:], h2ot_d[:])
        nc.sync.dma_start(h2ob_sb[:], h2ob_d[:])

        # Stage fp32 inputs through SBUF and round to float32r via DVE copies
        # (walrus requires fp32r matmul operands to come from a rounding op).
        with tc.tile_pool(name="stage", bufs=2) as stpool:
            CH = 4096
            for dst, src, width in ((batcht_r, batcht_d, n_steps * BL),
                                    (wg_r, wg_d, NCH * N),
                                    (i2ht_r, i2ht_d, N)):
                for j in range(0, width, CH):
                    w = min(CH, width - j)
                    st = stpool.tile([128, CH], DT.float32, tag="st")
                    nc.sync.dma_start(st[:, :w], src[:, j:j + w])
                    nc.vector.tensor_copy(dst[:, j:j + w], st[:, :w])
            st = stpool.tile([128, CH], DT.float32, tag="st")
            nc.sync.dma_start(st[:1, :N], crow_d[:])
            nc.vector.tensor_copy(crow_r[:], st[:1, :N])

        nc.vector.memset(ones_f[:], 1.0)
        nc.vector.tensor_copy(ones_r[:], ones_f[:])

        Y = spool.tile([128, NCH * BL], DT.float32r)     # y state (n-layout)
        W2 = spool.tile([128, NCH * BL], DT.float32)     # 0.1*x state
        tmp = spool.tile([128, NCH * BL], DT.float32)
        nc.vector.memset(W2[:], 0.0)
        nc.vector.memset(tmp[:], 0.0)

        for t in range(n_steps):
            a_ps = apool.tile([BL, N], DT.float32, tag="a")
            if t > 0:
                # tmp = 0.98*y - w   (runs while PE streams matmuls)
                nc.vector.scalar_tensor_tensor(
                    tmp[:], Y[:], 1.0 - H_STEP * 2.0 * GAMMA, W2[:],
                    op0=AL.mult, op1=AL.subtract)
            for h in range(2):
                cs = slice(512 * h, 512 * h + 512)
                nc.tensor.matmul(a_ps[:, cs], ones_r[:], crow_r[:, cs],
                                 start=True, stop=False)
                nc.tensor.matmul(a_ps[:, cs], batcht_r[:, t * BL:(t + 1) * BL],
                                 i2ht_r[:, cs], start=False, stop=(t == 0))
                if t > 0:
                    for c in range(NCH):
                        nc.tensor.matmul(
                            a_ps[:, cs], Y[:, 32 * c:32 * c + 32],
                            wg_r[:, N * c + 512 * h:N * c + 512 * h + 512],
                            start=False, stop=(c == NCH - 1))
            th0 = thpool.tile([BL, 512], DT.float32, tag="th0")
            th1 = thpool.tile([BL, 512], DT.float32, tag="th1")
            nc.scalar.activation(th0[:], a_ps[:, 0:512], TANH)
            nc.scalar.activation(th1[:], a_ps[:, 512:1024], TANH)
            thT = ttpool.tile([128, NCH * BL], DT.float32, tag="tt")
            for c in range(NCH):
                src = th0 if c < 4 else th1
                j = c % 4
                nc.tensor.matmul(thT[:, 32 * c:32 * c + 32],
                                 src[:, 128 * j:128 * j + 128], ident[:],
                                 is_transpose=True,
                                 start=(c == 0), stop=(c == NCH - 1))
            # y' = 0.1*tanh + tmp ; w' = 0.01*y' + w
            nc.vector.scalar_tensor_tensor(Y[:], thT[:], H_STEP, tmp[:],
                                           op0=AL.mult, op1=AL.add)
            nc.vector.scalar_tensor_tensor(W2[:], Y[:], H_STEP * H_STEP, W2[:],
                                           op0=AL.mult, op1=AL.add)

        out_ps = opool.tile([BL, O], DT.float32)
        for c in range(NCH):
            nc.tensor.matmul(out_ps[:], W2[:, 32 * c:32 * c + 32],
                             h2ot[:, O * c:O * c + O],
                             start=(c == 0), stop=False)
        nc.tensor.matmul(out_ps[:], ones_f[:], h2ob_sb[:],
                         start=False, stop=True)
        o_sb = spool.tile([BL, O], DT.float32)
        nc.scalar.copy(o_sb[:], out_ps[:])
        nc.sync.dma_start(out_d[:], o_sb[:])

    nc.compile()
    return nc


def host_prep(batch, i2h_w, i2h_b, h2h_w, h2h_b, h2o_w, h2o_b, n_steps=T):
    """Shared constants + per-core input maps (all layouts precomputed on host)."""
    g = np.float32(GAIN_REC)
    # moving operand for h2h: chunk c at cols [N*c, N*c+N); wg[p, N*c+n] = g*h2h_w[n, 128c+p]
    wg = np.ascontiguousarray(
        (g * h2h_w).T.reshape(NCH, 128, N).transpose(1, 0, 2).reshape(128, NCH * N))
    i2ht = np.ascontiguousarray(i2h_w.T)                      # [128, 1024]
    crow = (i2h_b + g * h2h_b).reshape(1, N).astype(np.float32)
    h2ot = np.ascontiguousarray(
        ((1.0 / H_STEP) * h2o_w).T.reshape(NCH, 128, O)
        .transpose(1, 0, 2).reshape(128, NCH * O)).astype(np.float32)
    h2ob = h2o_b.reshape(1, O).astype(np.float32)
    ident = np.eye(32, dtype=np.float32)

    shared = {"wg": wg.astype(np.float32), "i2ht": i2ht.astype(np.float32),
              "crow": crow, "h2ot": h2ot, "h2ob": h2ob, "ident": ident}
    in_maps = []
    for core in range(NCORES):
        bs = batch[:n_steps, core * BL:(core + 1) * BL, :]      # [T, 32, 128]
        batcht = np.ascontiguousarray(bs.transpose(2, 0, 1).reshape(I, n_steps * BL))
        in_maps.append({"batcht": batcht.astype(np.float32), **shared})
    return in_maps


def kernel(batch, i2h_w, i2h_b, h2h_w, h2h_b, h2o_w, h2o_b):
    if "nc" not in _cache:
        _cache["nc"] = build(T)
    nc = _cache["nc"]
    in_maps = host_prep(batch, i2h_w, i2h_b, h2h_w, h2h_b, h2o_w, h2o_b, T)
    res = bass_utils.run_bass_kernel_spmd(nc, in_maps, core_ids=list(range(NCORES)))
    out = np.empty((B, O), dtype=np.float32)
    for core in range(NCORES):
        out[core * BL:(core + 1) * BL] = res.results[core]["out"]
    return out


# revision 5
# speedup vs baseline: 1.0429x; 1.0429x over previous
"""HORN recurrent network kernel for 8x Trainium2 NeuronCores.

Model (T=512, B=256, I=128, N=1024, O=10):
    u_t  = i2h(batch_t)                          # input projection
    rec  = (1/sqrt(N)) * (y @ h2h_w.T + h2h_b)
    y'   = y + 0.1*(tanh(u_t + rec) - x - 0.2*y)
    x'   = x + 0.1*y'
    out  = x_T @ h2o_w.T + h2o_b

Sharding: data-parallel over batch, 32 per core; weights replicated.

Per-core implementation notes:
  - State kept in "n-layout" tiles [128, 256]: partition p / column 32*c+b
    holds element n = 128*c + p of batch-lane b.  The h2h matmul uses the
    state slices directly as the stationary operand (yT), streaming the
    (host-pre-laid-out) g*h2h_w.T as the moving operand in float32r
    (TF32-like: 11 mantissa bits, exact MAC) at 1 col/cycle.
  - The tanh argument is accumulated fully in PSUM per 512-wide bank from
    three matmul groups: a K=1 bias matmul (ones x (i2h_b + g*h2h_b)), the
    inlined input projection (batch kept SBUF-resident for all 512 steps,
    stationary [128,32] slice per step), and 8 h2h chunk matmuls.
  - ACT applies tanh straight out of PSUM (b-layout [32,512]); PE
    transposes the tanh output back to n-layout ([32,128] -> [128,32] x8
    into one PSUM bank, start/stop flags shared across the 8 transposes);
    DVE finishes the state update with 3 fused scalar_tensor_tensor ops.
  - x is carried as w = 0.1*x (fits the (in0 op0 scalar) op1 in1 op form);
    the final x = 10*w rescale is folded into the h2o weights host-side.
"""
import math
from contextlib import ExitStack

import numpy as np

import concourse.bacc as bacc
import concourse.bass_utils as bass_utils
import concourse.mybir as mybir
import concourse.tile as tile

T, B, I, N, O = 512, 256, 128, 1024, 10
H_STEP = 0.1
ALPHA = 1.0
OMEGA = 1.0
GAMMA = 0.1
GAIN_REC = 1.0 / math.sqrt(N)

NCORES = 8
BL = B // NCORES          # 32 batch lanes per core
NCH = N // 128            # 8 contraction chunks
DT = mybir.dt

_cache = {}


def build(n_steps=T):
    nc = bacc.Bacc("TRN2", target_bir_lowering=False, debug=False,
                   num_devices=NCORES)

    batcht_d = nc.dram_tensor("batcht", [I, n_steps * BL], DT.float32, kind="ExternalInput").ap()
    wg_d = nc.dram_tensor("wg", [128, NCH * N], DT.float32, kind="ExternalInput").ap()
    i2ht_d = nc.dram_tensor("i2ht", [I, N], DT.float32, kind="ExternalInput").ap()
    crow_d = nc.dram_tensor("crow", [1, N], DT.float32, kind="ExternalInput").ap()
    h2ot_d = nc.dram_tensor("h2ot", [128, NCH * O], DT.float32, kind="ExternalInput").ap()
    h2ob_d = nc.dram_tensor("h2ob", [1, O], DT.float32, kind="ExternalInput").ap()
    ident_d = nc.dram_tensor("ident", [32, 32], DT.float32, kind="ExternalInput").ap()
    out_d = nc.dram_tensor("out", [BL, O], DT.float32, kind="ExternalOutput").ap()

    AL = mybir.AluOpType
    TANH = mybir.ActivationFunctionType.Tanh

    with tile.TileContext(nc) as tc, ExitStack() as ctx:
        cpool = ctx.enter_context(tc.tile_pool(name="consts", bufs=1))
        spool = ctx.enter_context(tc.tile_pool(name="state", bufs=1))
        thpool = ctx.enter_context(tc.tile_pool(name="th", bufs=2))
        apool = ctx.enter_context(tc.tile_pool(name="aps", bufs=2, space="PSUM"))
        ttpool = ctx.enter_context(tc.tile_pool(name="tt", bufs=1, space="PSUM"))
        opool = ctx.enter_context(tc.tile_pool(name="ops", bufs=1, space="PSUM"))

        batcht_r = cpool.tile([I, n_steps * BL], DT.float32r)
        wg_r = cpool.tile([128, NCH * N], DT.float32r)
        i2ht_r = cpool.tile([I, N], DT.float32r)
        crow_r = cpool.tile([1, N], DT.float32r)
        ones_r = cpool.tile([1, BL], DT.float32r)
        ones_f = cpool.tile([1, BL], DT.float32)
        ident = cpool.tile([32, 32], DT.float32)
        h2ot = cpool.tile([128, NCH * O], DT.float32)
        h2ob_sb = cpool.tile([1, O], DT.float32)

        nc.sync.dma_start(ident[:], ident_d[:])
        nc.sync.dma_start(h2ot[:], h2ot_d[:])
        nc.sync.dma_start(h2ob_sb[:], h2ob_d[:])

        # Stage fp32 inputs through SBUF and round to float32r via DVE copies
        # (walrus requires fp32r matmul operands to come from a rounding op).
        with tc.tile_pool(name="stage", bufs=2) as stpool:
            CH = 4096
            for dst, src, width in ((batcht_r, batcht_d, n_steps * BL),
                                    (wg_r, wg_d, NCH * N),
                                    (i2ht_r, i2ht_d, N)):
                for j in range(0, width, CH):
                    w = min(CH, width - j)
                    st = stpool.tile([128, CH], DT.float32, tag="st")
                    nc.sync.dma_start(st[:, :w], src[:, j:j + w])
                    nc.vector.tensor_copy(dst[:, j:j + w], st[:, :w])
            st = stpool.tile([128, CH], DT.float32, tag="st")
            nc.sync.dma_start(st[:1, :N], crow_d[:])
            nc.vector.tensor_copy(crow_r[:], st[:1, :N])

        nc.vector.memset(ones_f[:], 1.0)
        nc.vector.tensor_copy(ones_r[:], ones_f[:])

        Y = spool.tile([128, NCH * BL], DT.float32r)     # y state (n-layout)
        W2 = spool.tile([128, NCH * BL], DT.float32)     # 0.1*x state
        tmp = spool.tile([128, NCH * BL], DT.float32)
        nc.vector.memset(W2[:], 0.0)
        nc.vector.memset(tmp[:], 0.0)

        # -------- software-pipelined recurrence --------
        # PE stream per step: ..., y-mms(t-1), c/u-mms(t)  [fills the tanh(t-1)
        # latency], transposes(t-1), y-mms(t), ...
        a_tiles = {}
        th_tiles = {}

        def emit_cu(t):
            a_ps = apool.tile([BL, N], DT.float32, tag="a", name=f"a{t}")
            a_tiles[t] = a_ps
            for h in range(2):
                cs = slice(512 * h, 512 * h + 512)
                nc.tensor.matmul(a_ps[:, cs], ones_r[:], crow_r[:, cs],
                                 start=True, stop=False)
                nc.tensor.matmul(a_ps[:, cs], batcht_r[:, t * BL:(t + 1) * BL],
                                 i2ht_r[:, cs], start=False, stop=(t == 0))

        def emit_y(t):
            a_ps = a_tiles[t]
            for h in range(2):
                cs = slice(512 * h, 512 * h + 512)
                for c in range(NCH):
                    nc.tensor.matmul(
                        a_ps[:, cs], Y[:, 32 * c:32 * c + 32],
                        wg_r[:, N * c + 512 * h:N * c + 512 * h + 512],
                        start=False, stop=(c == NCH - 1))

        def emit_tanh(t):
            a_ps = a_tiles.pop(t)
            th0 = thpool.tile([BL, 512], DT.float32, tag="th0", name=f"th0_{t}")
            th1 = thpool.tile([BL, 512], DT.float32, tag="th1", name=f"th1_{t}")
            th_tiles[t] = (th0, th1)
            nc.scalar.activation(th0[:], a_ps[:, 0:512], TANH)
            nc.scalar.activation(th1[:], a_ps[:, 512:1024], TANH)

        def emit_tail(t):
            # transposes (PE) + state update (DVE)
            th0, th1 = th_tiles.pop(t)
            ta = ttpool.tile([128, 128], DT.float32, tag="ta", name=f"ta{t}")
            tb = ttpool.tile([128, 128], DT.float32, tag="tb", name=f"tb{t}")
            for c in range(NCH):
                src, dst = (th0, ta) if c < 4 else (th1, tb)
                j = c % 4
                nc.tensor.matmul(dst[:, 32 * j:32 * j + 32],
                                 src[:, 128 * j:128 * j + 128], ident[:],
                                 is_transpose=True,
                                 start=(j == 0), stop=(j == 3))
                if c == 3:
                    # y'[0:128] ready as soon as the first 4 transposes land
                    nc.vector.scalar_tensor_tensor(
                        Y[:, 0:128], ta[:], H_STEP, tmp[:, 0:128],
                        op0=AL.mult, op1=AL.add)
            nc.vector.scalar_tensor_tensor(
                Y[:, 128:256], tb[:], H_STEP, tmp[:, 128:256],
                op0=AL.mult, op1=AL.add)
            nc.vector.scalar_tensor_tensor(W2[:], Y[:], H_STEP * H_STEP, W2[:],
                                           op0=AL.mult, op1=AL.add)

        def emit_stt1(s):
            # tmp = 0.98*y - w for step s (DVE, runs under the matmul stream)
            nc.vector.scalar_tensor_tensor(
                tmp[:], Y[:], 1.0 - H_STEP * 2.0 * GAMMA, W2[:],
                op0=AL.mult, op1=AL.subtract)

        emit_cu(0)
        for t in range(1, n_steps):
            s = t - 1
            if s > 0:
                emit_stt1(s)   # t=0 uses the memset tmp (x=y=0)
                emit_y(s)
            emit_tanh(s)
            emit_cu(t)
            emit_tail(s)
        s = n_steps - 1
        if s > 0:
            emit_stt1(s)
            emit_y(s)
        emit_tanh(s)
        emit_tail(s)

        out_ps = opool.tile([BL, O], DT.float32)
        for c in range(NCH):
            nc.tensor.matmul(out_ps[:], W2[:, 32 * c:32 * c + 32],
                             h2ot[:, O * c:O * c + O],
                             start=(c == 0), stop=False)
        nc.tensor.matmul(out_ps[:], ones_f[:], h2ob_sb[:],
                         start=False, stop=True)
        o_sb = spool.tile([BL, O], DT.float32)
        nc.scalar.copy(o_sb[:], out_ps[:])
        nc.sync.dma_start(out_d[:], o_sb[:])

    nc.compile()
    return nc


def host_prep(batch, i2h_w, i2h_b, h2h_w, h2h_b, h2o_w, h2o_b, n_steps=T):
    """Shared constants + per-core input maps (all layouts precomputed on host)."""
    g = np.float32(GAIN_REC)
    # moving operand for h2h: chunk c at cols [N*c, N*c+N); wg[p, N*c+n] = g*h2h_w[n, 128c+p]
    wg = np.ascontiguousarray(
        (g * h2h_w).T.reshape(NCH, 128, N).transpose(1, 0, 2).reshape(128, NCH * N))
    i2ht = np.ascontiguousarray(i2h_w.T)                      # [128, 1024]
    crow = (i2h_b + g * h2h_b).reshape(1, N).astype(np.float32)
    h2ot = np.ascontiguousarray(
        ((1.0 / H_STEP) * h2o_w).T.reshape(NCH, 128, O)
        .transpose(1, 0, 2).reshape(128, NCH * O)).astype(np.float32)
    h2ob = h2o_b.reshape(1, O).astype(np.float32)
    ident = np.eye(32, dtype=np.float32)

    shared = {"wg": wg.astype(np.float32), "i2ht": i2ht.astype(np.float32),
              "crow": crow, "h2ot": h2ot, "h2ob": h2ob, "ident": ident}
    in_maps = []
    for core in range(NCORES):
        bs = batch[:n_steps, core * BL:(core + 1) * BL, :]      # [T, 32, 128]
        batcht = np.ascontiguousarray(bs.transpose(2, 0, 1).reshape(I, n_steps * BL))
        in_maps.append({"batcht": batcht.astype(np.float32), **shared})
    return in_maps


def kernel(batch, i2h_w, i2h_b, h2h_w, h2h_b, h2o_w, h2o_b):
    if "nc" not in _cache:
        _cache["nc"] = build(T)
    nc = _cache["nc"]
    in_maps = host_prep(batch, i2h_w, i2h_b, h2h_w, h2h_b, h2o_w, h2o_b, T)
    res = bass_utils.run_bass_kernel_spmd(nc, in_maps, core_ids=list(range(NCORES)))
    out = np.empty((B, O), dtype=np.float32)
    for core in range(NCORES):
        out[core * BL:(core + 1) * BL] = res.results[core]["out"]
    return out


# revision 8
# speedup vs baseline: 1.1705x; 1.1223x over previous
"""HORN recurrent network kernel for 8x Trainium2 NeuronCores.

Model (T=512, B=256, I=128, N=1024, O=10):
    u_t  = i2h(batch_t)                          # input projection
    rec  = (1/sqrt(N)) * (y @ h2h_w.T + h2h_b)
    y'   = y + 0.1*(tanh(u_t + rec) - x - 0.2*y)
    x'   = x + 0.1*y'
    out  = x_T @ h2o_w.T + h2o_b

Sharding: data-parallel over batch, 32 per core; weights replicated.

Per-core implementation notes:
  - State kept in "n-layout" tiles [128, 256]: partition p / column 32*c+b
    holds element n = 128*c + p of batch-lane b.  The h2h matmul uses the
    state slices directly as the stationary operand (yT), streaming the
    (host-pre-laid-out) g*h2h_w.T as the moving operand in float32r
    (TF32-like: 11 mantissa bits, exact MAC) at 1 col/cycle.
  - The tanh argument is accumulated fully in PSUM per 512-wide bank from
    three matmul groups: a K=1 bias matmul (ones x (i2h_b + g*h2h_b)), the
    inlined input projection (batch kept SBUF-resident for all 512 steps,
    stationary [128,32] slice per step), and 8 h2h chunk matmuls.
  - ACT applies tanh straight out of PSUM (b-layout [32,512]); PE
    transposes the tanh output back to n-layout ([32,128] -> [128,32] x8
    into one PSUM bank, start/stop flags shared across the 8 transposes);
    DVE finishes the state update with 3 fused scalar_tensor_tensor ops.
  - x is carried as w = 0.1*x (fits the (in0 op0 scalar) op1 in1 op form);
    the final x = 10*w rescale is folded into the h2o weights host-side.
"""
import math
from contextlib import ExitStack

import numpy as np

import concourse.bacc as bacc
import concourse.bass_utils as bass_utils
import concourse.mybir as mybir
import concourse.tile as tile

T, B, I, N, O = 512, 256, 128, 1024, 10
H_STEP = 0.1
ALPHA = 1.0
OMEGA = 1.0
GAMMA = 0.1
GAIN_REC = 1.0 / math.sqrt(N)

NCORES = 8
BL = B // NCORES          # 32 batch lanes per core
NCH = N // 128            # 8 contraction chunks
DT = mybir.dt

_cache = {}


def build(n_steps=T):
    nc = bacc.Bacc("TRN2", target_bir_lowering=False, debug=False,
                   num_devices=NCORES)

    batcht_d = nc.dram_tensor("batcht", [I, n_steps * BL], DT.float32, kind="ExternalInput").ap()
    wg_d = nc.dram_tensor("wg", [128, NCH * N], DT.float32, kind="ExternalInput").ap()
    i2ht_d = nc.dram_tensor("i2ht", [I, N], DT.float32, kind="ExternalInput").ap()
    crow_d = nc.dram_tensor("crow", [1, N], DT.float32, kind="ExternalInput").ap()
    h2ot_d = nc.dram_tensor("h2ot", [128, NCH * O], DT.float32, kind="ExternalInput").ap()
    h2ob_d = nc.dram_tensor("h2ob", [1, O], DT.float32, kind="ExternalInput").ap()
    ident_d = nc.dram_tensor("ident", [32, 32], DT.float32, kind="ExternalInput").ap()
    out_d = nc.dram_tensor("out", [BL, O], DT.float32, kind="ExternalOutput").ap()

    AL = mybir.AluOpType
    TANH = mybir.ActivationFunctionType.Tanh

    with tile.TileContext(nc) as tc, ExitStack() as ctx:
        cpool = ctx.enter_context(tc.tile_pool(name="consts", bufs=1))
        spool = ctx.enter_context(tc.tile_pool(name="state", bufs=1))
        thpool = ctx.enter_context(tc.tile_pool(name="th", bufs=2))
        apool = ctx.enter_context(tc.tile_pool(name="aps", bufs=2, space="PSUM"))
        ttpool = ctx.enter_context(tc.tile_pool(name="tt", bufs=1, space="PSUM"))
        opool = ctx.enter_context(tc.tile_pool(name="ops", bufs=1, space="PSUM"))

        batcht_r = cpool.tile([I, n_steps * BL], DT.float32r)
        wg_r = cpool.tile([128, NCH * N], DT.float32r)
        i2ht_r = cpool.tile([I, N], DT.float32r)
        crow_r = cpool.tile([1, N], DT.float32r)
        ones_r = cpool.tile([1, BL], DT.float32r)
        ones_f = cpool.tile([1, BL], DT.float32)
        ident = cpool.tile([32, 32], DT.float32)
        h2ot = cpool.tile([128, NCH * O], DT.float32)
        h2ob_sb = cpool.tile([1, O], DT.float32)

        nc.sync.dma_start(ident[:], ident_d[:])
        nc.sync.dma_start(h2ot[:], h2ot_d[:])
        nc.sync.dma_start(h2ob_sb[:], h2ob_d[:])

        # Stage fp32 inputs through SBUF and round to float32r via DVE copies
        # (walrus requires fp32r matmul operands to come from a rounding op).
        with tc.tile_pool(name="stage", bufs=2) as stpool:
            CH = 4096
            for dst, src, width in ((batcht_r, batcht_d, n_steps * BL),
                                    (wg_r, wg_d, NCH * N),
                                    (i2ht_r, i2ht_d, N)):
                for j in range(0, width, CH):
                    w = min(CH, width - j)
                    st = stpool.tile([128, CH], DT.float32, tag="st")
                    nc.sync.dma_start(st[:, :w], src[:, j:j + w])
                    nc.vector.tensor_copy(dst[:, j:j + w], st[:, :w])
            st = stpool.tile([128, CH], DT.float32, tag="st")
            nc.sync.dma_start(st[:1, :N], crow_d[:])
            nc.vector.tensor_copy(crow_r[:], st[:1, :N])

        nc.vector.memset(ones_f[:], 1.0)
        nc.vector.tensor_copy(ones_r[:], ones_f[:])

        Y = spool.tile([128, NCH * BL], DT.float32r)     # y state (n-layout)
        W2 = spool.tile([128, NCH * BL], DT.float32)     # 0.1*x state
        tmp = spool.tile([128, NCH * BL], DT.float32)
        nc.vector.memset(W2[:], 0.0)
        nc.vector.memset(tmp[:], 0.0)

        # -------- software-pipelined recurrence --------
        # PE stream per step: ..., y-mms(t-1), c/u-mms(t)  [fills the tanh(t-1)
        # latency], transposes(t-1), y-mms(t), ...
        a_tiles = {}
        th_tiles = {}

        def emit_cu(t):
            a_ps = apool.tile([BL, N], DT.float32, tag="a", name=f"a{t}")
            a_tiles[t] = a_ps
            for h in range(2):
                cs = slice(512 * h, 512 * h + 512)
                nc.tensor.matmul(a_ps[:, cs], ones_r[:], crow_r[:, cs],
                                 start=True, stop=False)
            for h in range(2):
                cs = slice(512 * h, 512 * h + 512)
                nc.tensor.matmul(a_ps[:, cs], batcht_r[:, t * BL:(t + 1) * BL],
                                 i2ht_r[:, cs], start=False, stop=(t == 0))

        def emit_y(t):
            a_ps = a_tiles[t]
            for h in range(2):
                cs = slice(512 * h, 512 * h + 512)
                for c in range(NCH):
                    nc.tensor.matmul(
                        a_ps[:, cs], Y[:, 32 * c:32 * c + 32],
                        wg_r[:, N * c + 512 * h:N * c + 512 * h + 512],
                        start=False, stop=(c == NCH - 1))

        def emit_tanh(t):
            a_ps = a_tiles.pop(t)
            th0 = thpool.tile([BL, 512], DT.float32, tag="th0", name=f"th0_{t}")
            th1 = thpool.tile([BL, 512], DT.float32, tag="th1", name=f"th1_{t}")
            th_tiles[t] = (th0, th1)
            nc.scalar.activation(th0[:], a_ps[:, 0:512], TANH)
            nc.scalar.activation(th1[:], a_ps[:, 512:1024], TANH)

        def emit_tail(t):
            # transposes (PE) + state update (DVE).  One PSUM bank per
            # transpose-pair so each 64-wide y' slice updates as soon as its
            # pair lands -- by the time PE reaches the y-mms of the next step
            # the first chunks of Y are already written.
            th0, th1 = th_tiles.pop(t)
            for p in range(4):
                tp = ttpool.tile([128, 64], DT.float32, tag=f"tp{p}",
                                 name=f"tp{p}_{t}")
                for k in range(2):
                    c = 2 * p + k
                    src = th0 if c < 4 else th1
                    j = c % 4
                    nc.tensor.matmul(tp[:, 32 * k:32 * k + 32],
                                     src[:, 128 * j:128 * j + 128], ident[:],
                                     is_transpose=True,
                                     start=(k == 0), stop=(k == 1))
                nc.vector.scalar_tensor_tensor(
                    Y[:, 64 * p:64 * p + 64], tp[:], H_STEP,
                    tmp[:, 64 * p:64 * p + 64], op0=AL.mult, op1=AL.add)
            nc.vector.scalar_tensor_tensor(W2[:], Y[:], H_STEP * H_STEP, W2[:],
                                           op0=AL.mult, op1=AL.add)

        def emit_stt1(s):
            # tmp = 0.98*y - w for step s (DVE, runs under the matmul stream)
            nc.vector.scalar_tensor_tensor(
                tmp[:], Y[:], 1.0 - H_STEP * 2.0 * GAMMA, W2[:],
                op0=AL.mult, op1=AL.subtract)

        emit_cu(0)
        for t in range(1, n_steps):
            s = t - 1
            if s > 0:
                emit_stt1(s)   # t=0 uses the memset tmp (x=y=0)
                emit_y(s)
            emit_tanh(s)
            emit_cu(t)
            emit_tail(s)
        s = n_steps - 1
        if s > 0:
            emit_stt1(s)
            emit_y(s)
        emit_tanh(s)
        emit_tail(s)

        out_ps = apool.tile([BL, O], DT.float32, tag="a", name="out_ps")
        for c in range(NCH):
            nc.tensor.matmul(out_ps[:], W2[:, 32 * c:32 * c + 32],
                             h2ot[:, O * c:O * c + O],
                             start=(c == 0), stop=False)
        nc.tensor.matmul(out_ps[:], ones_f[:], h2ob_sb[:],
                         start=False, stop=True)
        o_sb = spool.tile([BL, O], DT.float32)
        nc.scalar.copy(o_sb[:], out_ps[:])
        nc.sync.dma_start(out_d[:], o_sb[:])

    nc.compile()
    return nc


def host_prep(batch, i2h_w, i2h_b, h2h_w, h2h_b, h2o_w, h2o_b, n_steps=T):
    """Shared constants + per-core input maps (all layouts precomputed on host)."""
    g = np.float32(GAIN_REC)
    # moving operand for h2h: chunk c at cols [N*c, N*c+N); wg[p, N*c+n] = g*h2h_w[n, 128c+p]
    wg = np.ascontiguousarray(
        (g * h2h_w).T.reshape(NCH, 128, N).transpose(1, 0, 2).reshape(128, NCH * N))
    i2ht = np.ascontiguousarray(i2h_w.T)                      # [128, 1024]
    crow = (i2h_b + g * h2h_b).reshape(1, N).astype(np.float32)
    h2ot = np.ascontiguousarray(
        ((1.0 / H_STEP) * h2o_w).T.reshape(NCH, 128, O)
        .transpose(1, 0, 2).reshape(128, NCH * O)).astype(np.float32)
    h2ob = h2o_b.reshape(1, O).astype(np.float32)
    ident = np.eye(32, dtype=np.float32)

    shared = {"wg": wg.astype(np.float32), "i2ht": i2ht.astype(np.float32),
              "crow": crow, "h2ot": h2ot, "h2ob": h2ob, "ident": ident}
    in_maps = []
    for core in range(NCORES):
        bs = batch[:n_steps, core * BL:(core + 1) * BL, :]      # [T, 32, 128]
        batcht = np.ascontiguousarray(bs.transpose(2, 0, 1).reshape(I, n_steps * BL))
        in_maps.append({"batcht": batcht.astype(np.float32), **shared})
    return in_maps


def kernel(batch, i2h_w, i2h_b, h2h_w, h2h_b, h2o_w, h2o_b):
    if "nc" not in _cache:
        _cache["nc"] = build(T)
    nc = _cache["nc"]
    in_maps = host_prep(batch, i2h_w, i2h_b, h2h_w, h2h_b, h2o_w, h2o_b, T)
    res = bass_utils.run_bass_kernel_spmd(nc, in_maps, core_ids=list(range(NCORES)))
    out = np.empty((B, O), dtype=np.float32)
    for core in range(NCORES):
        out[core * BL:(core + 1) * BL] = res.results[core]["out"]
    return out


# revision 19
# speedup vs baseline: 1.2735x; 1.0881x over previous
"""HORN recurrent network kernel for 8x Trainium2 NeuronCores.

Model (T=512, B=256, I=128, N=1024, O=10):
    u_t  = i2h(batch_t)                          # input projection
    rec  = (1/sqrt(N)) * (y @ h2h_w.T + h2h_b)
    y'   = y + 0.1*(tanh(u_t + rec) - x - 0.2*y)
    x'   = x + 0.1*y'
    out  = x_T @ h2o_w.T + h2o_b

Sharding: data-parallel over batch, 32 per core; weights replicated.

Per-core implementation notes:
  - State kept in "n-layout" tiles [128, 256]: partition p / column 32*c+b
    holds element n = 128*c + p of batch-lane b.  The h2h matmul uses the
    state slices directly as the stationary operand (yT), streaming the
    (host-pre-laid-out) g*h2h_w.T as the moving operand in float32r
    (TF32-like: 11 mantissa bits, exact MAC) at 1 col/cycle.
  - The tanh argument is accumulated fully in PSUM per 512-wide bank from
    three matmul groups: a K=1 bias matmul (ones x (i2h_b + g*h2h_b)), the
    inlined input projection (batch kept SBUF-resident for all 512 steps,
    stationary [128,32] slice per step), and 8 h2h chunk matmuls.
  - ACT applies tanh straight out of PSUM (b-layout [32,512]); PE
    transposes the tanh output back to n-layout ([32,128] -> [128,32] x8
    into one PSUM bank, start/stop flags shared across the 8 transposes);
    DVE finishes the state update with 3 fused scalar_tensor_tensor ops.
  - x is carried as w = 0.1*x (fits the (in0 op0 scalar) op1 in1 op form);
    the final x = 10*w rescale is folded into the h2o weights host-side.
"""
import math
from contextlib import ExitStack

import numpy as np

import concourse.bacc as bacc
import concourse.bass_utils as bass_utils
import concourse.mybir as mybir
import concourse.tile as tile

T, B, I, N, O = 512, 256, 128, 1024, 10
H_STEP = 0.1
ALPHA = 1.0
OMEGA = 1.0
GAMMA = 0.1
GAIN_REC = 1.0 / math.sqrt(N)

NCORES = 8
BL = B // NCORES          # 32 batch lanes per core
NCH = N // 128            # 8 contraction chunks
DT = mybir.dt

# fp8 DoubleRow for the h2h matmuls: y scaled by SY and g*h2h_w.T by SW into
# fp8e4m3 (keeps values in the normal range); the combined factor
# F = SY*SW/GAIN_REC is applied to the input-projection/bias operands and
# removed for free via tanh's scale immediate.
SY = 8.0
SW = 8.0
F_SCALE = SY * SW / GAIN_REC          # 2048
NPAIR = NCH // 2                       # 4 doubled-K chunk pairs

_cache = {}


def build(n_steps=T):
    nc = bacc.Bacc("TRN2", target_bir_lowering=False, debug=False,
                   num_devices=NCORES)

    batcht_d = nc.dram_tensor("batcht", [I, n_steps * BL], DT.float32, kind="ExternalInput").ap()
    wg_d = nc.dram_tensor("wg", [128, NCH * N], DT.float8e4, kind="ExternalInput").ap()
    i2ht_d = nc.dram_tensor("i2ht", [I, N], DT.float32, kind="ExternalInput").ap()
    crow_d = nc.dram_tensor("crow", [1, N], DT.float32, kind="ExternalInput").ap()
    h2ot_d = nc.dram_tensor("h2ot", [128, NCH * O], DT.float32, kind="ExternalInput").ap()
    h2ob_d = nc.dram_tensor("h2ob", [1, O], DT.float32, kind="ExternalInput").ap()
    ident_d = nc.dram_tensor("ident", [32, 32], DT.float32, kind="ExternalInput").ap()
    out_d = nc.dram_tensor("out", [BL, O], DT.float32, kind="ExternalOutput").ap()

    AL = mybir.AluOpType
    TANH = mybir.ActivationFunctionType.Tanh

    with tile.TileContext(nc) as tc, ExitStack() as ctx:
        cpool = ctx.enter_context(tc.tile_pool(name="consts", bufs=1))
        spool = ctx.enter_context(tc.tile_pool(name="state", bufs=1))
        thpool = ctx.enter_context(tc.tile_pool(name="th", bufs=2))
        apool = ctx.enter_context(tc.tile_pool(name="aps", bufs=2, space="PSUM"))
        ttpool = ctx.enter_context(tc.tile_pool(name="tt", bufs=1, space="PSUM"))
        opool = ctx.enter_context(tc.tile_pool(name="ops", bufs=1, space="PSUM"))

        batcht_r = cpool.tile([I, n_steps * BL], DT.float32r)
        wg8 = cpool.tile([128, NCH * N], DT.float8e4)
        i2ht_r = cpool.tile([I, N], DT.float32r)
        crow_r = cpool.tile([1, N], DT.float32r)
        ones_r = cpool.tile([1, BL], DT.float32r)
        ones_f = cpool.tile([1, BL], DT.float32)
        ident = cpool.tile([32, 32], DT.float32)
        h2ot = cpool.tile([128, NCH * O], DT.float32)
        h2ob_sb = cpool.tile([1, O], DT.float32)

        nc.sync.dma_start(ident[:], ident_d[:])
        nc.sync.dma_start(h2ot[:], h2ot_d[:])
        nc.sync.dma_start(h2ob_sb[:], h2ob_d[:])
        nc.sync.dma_start(wg8[:], wg_d[:])

        # Stage fp32 inputs through SBUF and round to float32r via DVE copies
        # (walrus requires fp32r matmul operands to come from a rounding op).
        with tc.tile_pool(name="stage", bufs=2) as stpool:
            CH = 4096
            for dst, src, width in ((batcht_r, batcht_d, n_steps * BL),
                                    (i2ht_r, i2ht_d, N)):
                for j in range(0, width, CH):
                    w = min(CH, width - j)
                    st = stpool.tile([128, CH], DT.float32, tag="st")
                    nc.sync.dma_start(st[:, :w], src[:, j:j + w])
                    nc.vector.tensor_copy(dst[:, j:j + w], st[:, :w])
            st = stpool.tile([128, CH], DT.float32, tag="st")
            nc.sync.dma_start(st[:1, :N], crow_d[:])
            nc.vector.tensor_copy(crow_r[:], st[:1, :N])

        nc.vector.memset(ones_f[:], 1.0)
        nc.vector.tensor_copy(ones_r[:], ones_f[:])

        Y = spool.tile([128, NCH * BL], DT.float32)      # y state (n-layout)
        Y8 = spool.tile([128, NCH * BL], DT.float8e4)    # SY*y, h2h stationary
        W2 = spool.tile([128, NCH * BL], DT.float32)     # 0.1*x state
        tmp = spool.tile([128, NCH * BL], DT.float32)
        nc.vector.memset(W2[:], 0.0)
        nc.vector.memset(tmp[:], 0.0)
        wg8_4d = wg8[:].rearrange("p (cc two n) -> p cc two n", cc=NPAIR, two=2)

        # -------- software-pipelined recurrence --------
        # PE stream per step: ..., y-mms(t-1), c/u-mms(t)  [fills the tanh(t-1)
        # latency], transposes(t-1), y-mms(t), ...
        a_tiles = {}
        th_tiles = {}

        def emit_cu(t):
            a_ps = apool.tile([BL, N], DT.float32, tag="a", name=f"a{t}")
            a_tiles[t] = a_ps
            for h in range(2):
                cs = slice(512 * h, 512 * h + 512)
                nc.tensor.matmul(a_ps[:, cs], ones_r[:], crow_r[:, cs],
                                 start=True, stop=False)
            for h in range(2):
                cs = slice(512 * h, 512 * h + 512)
                nc.tensor.matmul(a_ps[:, cs], batcht_r[:, t * BL:(t + 1) * BL],
                                 i2ht_r[:, cs], start=False, stop=(t == 0))

        def emit_y(t):
            a_ps = a_tiles[t]
            for h in range(2):
                cs = slice(512 * h, 512 * h + 512)
                for cc in range(NPAIR):
                    lhsT = Y8[:, 64 * cc:64 * cc + 64].rearrange(
                        "p (two m) -> p two m", two=2)
                    nc.tensor.matmul(
                        a_ps[:, cs], lhsT,
                        wg8_4d[:, cc, :, 512 * h:512 * h + 512],
                        start=False, stop=(cc == NPAIR - 1),
                        perf_mode=mybir.MatmulPerfMode.DoubleRow)

        def emit_tanh(t):
            a_ps = a_tiles.pop(t)
            th0 = thpool.tile([BL, 512], DT.float32, tag="th0", name=f"th0_{t}")
            th1 = thpool.tile([BL, 512], DT.float32, tag="th1", name=f"th1_{t}")
            th_tiles[t] = (th0, th1)
            nc.scalar.activation(th0[:], a_ps[:, 0:512], TANH, scale=1.0 / F_SCALE)
            nc.scalar.activation(th1[:], a_ps[:, 512:1024], TANH, scale=1.0 / F_SCALE)

        def emit_tail(t):
            # transposes (PE) + state update (DVE).  One PSUM bank per
            # transpose-pair so each 64-wide y' slice updates as soon as its
            # pair lands -- by the time PE reaches the y-mms of the next step
            # the first chunks of Y are already written.
            th0, th1 = th_tiles.pop(t)
            for p in range(4):
                tp = ttpool.tile([128, 64], DT.float32, tag=f"tp{p}",
                                 name=f"tp{p}_{t}")
                for k in range(2):
                    c = 2 * p + k
                    src = th0 if c < 4 else th1
                    j = c % 4
                    nc.tensor.matmul(tp[:, 32 * k:32 * k + 32],
                                     src[:, 128 * j:128 * j + 128], ident[:],
                                     is_transpose=True,
                                     start=(k == 0), stop=(k == 1))
                nc.vector.scalar_tensor_tensor(
                    Y[:, 64 * p:64 * p + 64], tp[:], H_STEP,
                    tmp[:, 64 * p:64 * p + 64], op0=AL.mult, op1=AL.add)
                if p == 1:
                    nc.vector.tensor_scalar_mul(Y8[:, 0:128], Y[:, 0:128], SY)
            nc.vector.tensor_scalar_mul(Y8[:, 128:256], Y[:, 128:256], SY)
            nc.vector.scalar_tensor_tensor(W2[:], Y[:], H_STEP * H_STEP, W2[:],
                                           op0=AL.mult, op1=AL.add)

        def emit_stt1(s):
            # tmp = 0.98*y - w for step s (DVE, runs under the matmul stream)
            nc.vector.scalar_tensor_tensor(
                tmp[:], Y[:], 1.0 - H_STEP * 2.0 * GAMMA, W2[:],
                op0=AL.mult, op1=AL.subtract)

        emit_cu(0)
        for t in range(1, n_steps):
            s = t - 1
            if s > 0:
                emit_stt1(s)   # t=0 uses the memset tmp (x=y=0)
                emit_y(s)
            emit_tanh(s)
            emit_cu(t)
            emit_tail(s)
        s = n_steps - 1
        if s > 0:
            emit_stt1(s)
            emit_y(s)
        emit_tanh(s)
        emit_tail(s)

        out_ps = apool.tile([BL, O], DT.float32, tag="a", name="out_ps")
        for c in range(NCH):
            nc.tensor.matmul(out_ps[:], W2[:, 32 * c:32 * c + 32],
                             h2ot[:, O * c:O * c + O],
                             start=(c == 0), stop=False)
        nc.tensor.matmul(out_ps[:], ones_f[:], h2ob_sb[:],
                         start=False, stop=True)
        o_sb = spool.tile([BL, O], DT.float32)
        nc.scalar.copy(o_sb[:], out_ps[:])
        nc.sync.dma_start(out_d[:], o_sb[:])

    nc.compile()
    return nc


def host_prep(batch, i2h_w, i2h_b, h2h_w, h2h_b, h2o_w, h2o_b, n_steps=T):
    """Shared constants + per-core input maps (all layouts precomputed on host)."""
    import ml_dtypes
    g = np.float32(GAIN_REC)
    # h2h moving operand, fp8 DoubleRow layout: col (cc, slot, n) holds
    # SW*h2h_w[n, k] for k = 256*cc + 128*slot + p
    wg = np.ascontiguousarray(
        (SW * h2h_w).T.reshape(NPAIR, 2, 128, N).transpose(2, 0, 1, 3)
        .reshape(128, NCH * N)).astype(ml_dtypes.float8_e4m3)
    i2ht = np.ascontiguousarray(F_SCALE * i2h_w.T)            # [128, 1024]
    crow = (F_SCALE * (i2h_b + g * h2h_b)).reshape(1, N).astype(np.float32)
    h2ot = np.ascontiguousarray(
        ((1.0 / H_STEP) * h2o_w).T.reshape(NCH, 128, O)
        .transpose(1, 0, 2).reshape(128, NCH * O)).astype(np.float32)
    h2ob = h2o_b.reshape(1, O).astype(np.float32)
    ident = np.eye(32, dtype=np.float32)

    shared = {"wg": wg, "i2ht": i2ht.astype(np.float32),
              "crow": crow, "h2ot": h2ot, "h2ob": h2ob, "ident": ident}
    in_maps = []
    for core in range(NCORES):
        bs = batch[:n_steps, core * BL:(core + 1) * BL, :]      # [T, 32, 128]
        batcht = np.ascontiguousarray(bs.transpose(2, 0, 1).reshape(I, n_steps * BL))
        in_maps.append({"batcht": batcht.astype(np.float32), **shared})
    return in_maps


def kernel(batch, i2h_w, i2h_b, h2h_w, h2h_b, h2o_w, h2o_b):
    if "nc" not in _cache:
        _cache["nc"] = build(T)
    nc = _cache["nc"]
    in_maps = host_prep(batch, i2h_w, i2h_b, h2h_w, h2h_b, h2o_w, h2o_b, T)
    res = bass_utils.run_bass_kernel_spmd(nc, in_maps, core_ids=list(range(NCORES)))
    out = np.empty((B, O), dtype=np.float32)
    for core in range(NCORES):
        out[core * BL:(core + 1) * BL] = res.results[core]["out"]
    return out
